# revision 31
# baseline (speedup 1.0000x reference)
# Multi-headed attention with additive tree/leaf score biases, on 8 TRN2
# NeuronCores. Data-parallel over batch: core b computes batch element b.
#
# Per-core dataflow (L=1024, D=512, H=8, dk=64), fp16 matmul operands with
# fp32 PSUM accumulation:
#   QTz_h[d,l]: zero-padded per-head Q^T (head rows live, other half 0) so
#       the QK matmul runs K=128 (fast-weight-load path) with the packed
#       two-head K^T tile as stationary operand.
#   ST[kpos,q] = KT_pair^T @ QTz_h    (scores transposed, pre-scaled 1/8)
#   e = exp(ST)  (ACT, psum->sbuf, fp16)
#   p = e * expBT,  expBT = exp(tree+leaf+maskbias - C)^T  (fp16, C=offset
#       keeping p in fp16 range; cancels in softmax normalization)
#   U[q, 0:64|64] = p^T @ [V_h | 1]   (col 64 = softmax denominator s)
#   ctx[q, 64h:64h+64] = U[:, 0:64] * (1/s)   (per-partition scalar)
#   out = ctx @ Wo^T (+bo)  via ctx^T on-device transpose
import numpy as np

B, L, D, H = 8, 1024, 512, 8
DK = D // H          # 64
P = 128
NQ = L // P          # 8 q-chunks
NK = L // P          # 8 kpos-chunks
KC = D // P          # 4 contraction chunks
MASK_C = 60.0        # masked-score offset (expB underflows to 0 in fp16)
EXPB_C = 6.931472    # ~10*ln(2): headroom so p = e*expB stays in fp16 range

_CACHE = {}


def _build(zb):
    import concourse.mybir as mybir
    import concourse.tile as tile
    from concourse import bacc

    f32 = mybir.dt.float32
    f16 = mybir.dt.float16
    i8 = mybir.dt.int8
    Exp = mybir.ActivationFunctionType.Exp
    Copy = mybir.ActivationFunctionType.Copy
    Ident = mybir.ActivationFunctionType.Identity
    Alu = mybir.AluOpType

    nc = bacc.Bacc("TRN2", target_bir_lowering=False, debug=False)

    xqT = nc.declare_dram_parameter("xqT", [D, L], f16, isOutput=False)
    xkT = nc.declare_dram_parameter("xkT", [D, L], f16, isOutput=False)
    xvT = nc.declare_dram_parameter("xvT", [D, L], f16, isOutput=False)
    wqT = nc.declare_dram_parameter("wqT", [D, D], f16, isOutput=False)
    wkT = nc.declare_dram_parameter("wkT", [D, D], f16, isOutput=False)
    wvT = nc.declare_dram_parameter("wvT", [D, D], f16, isOutput=False)
    woT = nc.declare_dram_parameter("woT", [D, D], f16, isOutput=False)
    bqc = nc.declare_dram_parameter("bqc", [D, 1], f32, isOutput=False)
    bkc = nc.declare_dram_parameter("bkc", [D, 1], f32, isOutput=False)
    bvd = nc.declare_dram_parameter("bv", [1, D], f16, isOutput=False)
    bodf = nc.declare_dram_parameter("bo", [1, D], f32, isOutput=False)
    mask = nc.declare_dram_parameter("mask", [L, L], i8, isOutput=False)
    tree = nc.declare_dram_parameter("tree", [L, L], f16, isOutput=False)
    leaf = nc.declare_dram_parameter("leaf", [L, L], f16, isOutput=False)
    identh = nc.declare_dram_parameter("identh", [P, P], f16, isOutput=False)
    ones = nc.declare_dram_parameter("ones", [1, D], f16, isOutput=False)
    out = nc.declare_dram_parameter("out", [L, D], f32, isOutput=True)

    with tile.TileContext(nc) as tc:
        import contextlib
        ctx = contextlib.ExitStack()
        with ctx:
            consts = ctx.enter_context(tc.tile_pool(name="consts", bufs=1))
            persist = ctx.enter_context(tc.tile_pool(name="persist", bufs=1))
            wpool = ctx.enter_context(tc.tile_pool(name="wpool", bufs=12))
            xpool = ctx.enter_context(tc.tile_pool(name="xpool", bufs=12))
            ep = ctx.enter_context(tc.tile_pool(name="ep", bufs=17))
            big4 = ctx.enter_context(tc.tile_pool(name="big4", bufs=4))
            bnat = ctx.enter_context(tc.tile_pool(name="bnat", bufs=2))
            small = ctx.enter_context(tc.tile_pool(name="small", bufs=4))
            outp = ctx.enter_context(tc.tile_pool(name="outp", bufs=3))
            ps_mm = ctx.enter_context(
                tc.tile_pool(name="ps_mm", bufs=3, space="PSUM"))
            ps_att = ctx.enter_context(
                tc.tile_pool(name="ps_att", bufs=2, space="PSUM"))

            # ---- constants ----
            t_identh = consts.tile([P, P], f16)
            nc.sync.dma_start(out=t_identh[:, :], in_=identh[:, :])
            t_ones = consts.tile([1, D], f16)
            nc.sync.dma_start(out=t_ones[:, :], in_=ones[:, :])
            if not zb:
                t_bvb = consts.tile([P, D], f16)
                nc.sync.dma_start(out=t_bvb[:, :],
                                  in_=bvd[:, :].broadcast_to([P, D]))
                t_bob = consts.tile([P, D], f32)
                nc.sync.dma_start(out=t_bob[:, :],
                                  in_=bodf[:, :].broadcast_to([P, D]))
                # per-partition bias columns for the Q/K projection evacs;
                # bq pre-divided by 8 (the evac applies scale=1/8 to psum)
                t_bqc, t_bkc = [], []
                for dc in range(KC):
                    tq = consts.tile([P, 1], f32, name=f"bqc_{dc}")
                    nc.sync.dma_start(out=tq[:, :],
                                      in_=bqc[dc * P:(dc + 1) * P, :])
                    tq8 = consts.tile([P, 1], f32, name=f"bqc8_{dc}")
                    nc.vector.tensor_scalar_mul(tq8[:, :], tq[:, :], 1.0 / 8.0)
                    t_bqc.append(tq8)
                    tk = consts.tile([P, 1], f32, name=f"bkc_{dc}")
                    nc.sync.dma_start(out=tk[:, :],
                                      in_=bkc[dc * P:(dc + 1) * P, :])
                    t_bkc.append(tk)

            # ---- weights/activations arrive fp16 from host ----
            def load_w_persist(dram):
                ts = []
                for kc in range(KC):
                    th = persist.tile([P, D], f16, tag=f"woT_{kc}",
                                      name=f"wh_{dram.name}_{kc}")
                    nc.sync.dma_start(
                        out=th[:, :], in_=dram[kc * P:(kc + 1) * P, :])
                    ts.append(th)
                return ts

            def load_w(dram, pool, tag):
                ts = []
                for kc in range(KC):
                    th = pool.tile([P, D], f16, tag=tag,
                                   name=f"wh_{dram.name}_{kc}")
                    nc.sync.dma_start(
                        out=th[:, :], in_=dram[kc * P:(kc + 1) * P, :])
                    ts.append(th)
                return ts

            def load_xT(dram):
                ts = []
                for kc in range(KC):
                    th = xpool.tile([P, L], f16, tag="xp",
                                    name=f"xh_{dram.name}_{kc}")
                    nc.sync.dma_start(
                        out=th[:, :], in_=dram[kc * P:(kc + 1) * P, :])
                    ts.append(th)
                return ts

            # ---- bias matrix pipeline (after QK: attention needs it a bit later):
            #      B = tree + leaf + (mask==0 ? -MASK_C : 0) - EXPB_C
            #      expBT_all[:, kc*L + q] = exp(B)[q, kc*128 + p]  (fp16) ----
            t_expBT = persist.tile([P, NK * L], f16, tag="expBT", name="expBT")
            for qc in range(NQ):
                bn = bnat.tile([P, L], f16, tag="bn", name="bn")
                nc.sync.dma_start(out=bn[:, :], in_=tree[qc * P:(qc + 1) * P, :])
                lf = bnat.tile([P, L], f16, tag="lf", name="lf")
                nc.sync.dma_start(out=lf[:, :], in_=leaf[qc * P:(qc + 1) * P, :])
                mk = bnat.tile([P, L], i8, tag="mk", name="mk")
                nc.sync.dma_start(out=mk[:, :], in_=mask[qc * P:(qc + 1) * P, :])
                mb = bnat.tile([P, L], f16, tag="mb", name="mb")
                nc.gpsimd.tensor_scalar(
                    out=mb[:, :], in0=mk[:, :], scalar1=MASK_C,
                    scalar2=-(MASK_C + EXPB_C), op0=Alu.mult, op1=Alu.add)
                bs = bnat.tile([P, L], f16, tag="bs", name="bs")
                nc.vector.tensor_tensor(
                    out=bs[:, :], in0=bn[:, :], in1=lf[:, :], op=Alu.add)
                nc.vector.tensor_tensor(
                    out=bs[:, :], in0=bs[:, :], in1=mb[:, :], op=Alu.add)
                eb = bnat.tile([P, L], f16, tag="eb", name="eb")
                nc.scalar.activation(out=eb[:, :], in_=bs[:, :], func=Exp)
                # transpose eb into column-slab qc of expBT (per kc chunk)
                pst = ps_mm.tile([P, L], f16, tag="ps_mm", name="ps_bt")
                for kc in range(NK):
                    nc.tensor.transpose(
                        out=pst[:, kc * P:(kc + 1) * P],
                        in_=eb[:, kc * P:(kc + 1) * P],
                        identity=t_identh[:, :])
                nc.vector.tensor_copy(
                    t_expBT[:, :].rearrange("p (kc q) -> p kc q", q=L)
                    [:, :, qc * P:(qc + 1) * P],
                    pst[:, :].rearrange("p (kc c) -> p kc c", c=P))


            t_wqT = load_w(wqT, wpool, "wp")
            t_wkT = load_w(wkT, wpool, "wp")
            t_xqT = load_xT(xqT)
            t_xkT = load_xT(xkT)

            # ---- Q/K projections.  K^T packed two heads per tile;
            #      Q^T zero-padded per head (K=128 QK matmuls -> FWL). ----
            t_QTz = []
            for h in range(H):
                qz = persist.tile([P, L], f16, tag=f"qTz_{h}", name=f"qTz_{h}")
                t_QTz.append(qz)
                if h % 2 == 0:
                    nc.gpsimd.memset(qz[DK:P, :], 0.0)
                else:
                    nc.gpsimd.memset(qz[0:DK, :], 0.0)
            t_KT = []
            for dc in range(KC):
                kt = persist.tile([P, L], f16, tag=f"kT_{dc}", name=f"kT_{dc}")
                t_KT.append(kt)
                for lh in range(2):
                    ps = ps_mm.tile([P, D], f32, tag="ps_mm", name="ps_k")
                    for kc in range(KC):
                        nc.tensor.matmul(
                            ps[:, :],
                            lhsT=t_wkT[kc][:, dc * P:(dc + 1) * P],
                            rhs=t_xkT[kc][:, lh * D:(lh + 1) * D],
                            start=(kc == 0), stop=(kc == KC - 1))
                    if zb:
                        nc.vector.tensor_copy(
                            kt[:, lh * D:(lh + 1) * D], ps[:, :])
                    else:
                        nc.scalar.activation(
                            out=kt[:, lh * D:(lh + 1) * D], in_=ps[:, :],
                            func=Ident, bias=t_bkc[dc][:, :])
            for dc in range(KC):
                for lh in range(2):
                    ps = ps_mm.tile([P, D], f32, tag="ps_mm", name="ps_q")
                    for kc in range(KC):
                        nc.tensor.matmul(
                            ps[:, :],
                            lhsT=t_wqT[kc][:, dc * P:(dc + 1) * P],
                            rhs=t_xqT[kc][:, lh * D:(lh + 1) * D],
                            start=(kc == 0), stop=(kc == KC - 1))
                    # psum rows 0:64 -> head 2dc, rows 64:128 -> head 2dc+1
                    if zb:
                        nc.scalar.activation(
                            out=t_QTz[2 * dc][0:DK, lh * D:(lh + 1) * D],
                            in_=ps[0:DK, :], func=Copy, scale=1.0 / 8.0)
                        nc.scalar.activation(
                            out=t_QTz[2 * dc + 1][DK:P, lh * D:(lh + 1) * D],
                            in_=ps[DK:P, :], func=Copy, scale=1.0 / 8.0)
                    else:
                        nc.scalar.activation(
                            out=t_QTz[2 * dc][0:DK, lh * D:(lh + 1) * D],
                            in_=ps[0:DK, :], func=Ident, scale=1.0 / 8.0,
                            bias=t_bqc[dc][0:DK, :])
                        nc.scalar.activation(
                            out=t_QTz[2 * dc + 1][DK:P, lh * D:(lh + 1) * D],
                            in_=ps[DK:P, :], func=Ident, scale=1.0 / 8.0,
                            bias=t_bqc[dc][DK:P, :])

            # ---- Xv^T + V projection -> V_aug [128, 520] fp16 x 8 ----
            t_wvT = load_w(wvT, wpool, "wp")
            t_xvT = load_xT(xvT)
            t_woT = load_w_persist(woT)
            t_vaug = []
            for lc in range(NK):
                va = persist.tile([P, H * (DK + 1)], f16, tag=f"vaug_{lc}",
                                  name=f"vaug_{lc}")
                t_vaug.append(va)
                nc.gpsimd.memset(va[:, :], 1.0)
                ps = ps_mm.tile([P, D], f32, tag="ps_mm", name="ps_v")
                for kc in range(KC):
                    nc.tensor.matmul(
                        ps[:, :],
                        lhsT=t_xvT[kc][:, lc * P:(lc + 1) * P],
                        rhs=t_wvT[kc][:, :],
                        start=(kc == 0), stop=(kc == KC - 1))
                if zb:
                    nc.vector.tensor_copy(
                        va[:, :].rearrange("p (h c) -> p h c", c=DK + 1)[:, :, 0:DK],
                        ps[:, :].rearrange("p (h c) -> p h c", c=DK))
                else:
                    nc.scalar.activation(
                        out=va[:, :].rearrange("p (h c) -> p h c", c=DK + 1)[:, :, 0:DK],
                        in_=ps[:, :].rearrange("p (h c) -> p h c", c=DK),
                        func=Copy)
                    nc.vector.tensor_tensor(
                        out=va[:, :].rearrange("p (h c) -> p h c", c=DK + 1)[:, :, 0:DK],
                        in0=va[:, :].rearrange("p (h c) -> p h c", c=DK + 1)[:, :, 0:DK],
                        in1=t_bvb[:, :].rearrange("p (h c) -> p h c", c=DK),
                        op=Alu.add)

            # ---- attention: per head, kc-incremental attnV ----
            t_ctx = [persist.tile([P, D], f16, tag=f"ctx_{qc}",
                                  name=f"ctx_{qc}") for qc in range(NQ)]
            t_cT = [persist.tile([P, D], f16, tag=f"cT_{qc}",
                                 name=f"cT_{qc}") for qc in range(NQ)]
            t_acc = []
            pending_ct = []
            W65 = DK + 1
            for h in range(H):
                ht = h // 2
                pso = [ps_att.tile([P, 4 * W65], f32, tag="ps_att", name="pso")
                       for _ in range(2)]
                for kc in range(NK):
                    if kc == 4 and pending_ct:
                        pt = pending_ct.pop(0)
                        for qc in range(NQ):
                            pct = ps_att.tile([P, P], f16, tag="ps_att",
                                              name="pct")
                            nc.tensor.transpose(
                                out=pct[:, :],
                                in_=t_ctx[qc][:, pt * P:(pt + 1) * P],
                                identity=t_identh[:, :])
                            nc.vector.tensor_copy(
                                t_cT[qc][:, pt * P:(pt + 1) * P], pct[:, :])
                    ps = ps_mm.tile([P, L], f32, tag="ps_mm", name="ps_sc")
                    for qh in range(2):
                        nc.tensor.matmul(
                            ps[:, qh * D:(qh + 1) * D],
                            lhsT=t_KT[ht][:, kc * P:(kc + 1) * P],
                            rhs=t_QTz[h][:, qh * D:(qh + 1) * D],
                            start=True, stop=True)
                    e = ep.tile([P, L], f16, tag="ep", name="e_t")
                    nc.scalar.activation(out=e[:, :], in_=ps[:, :], func=Exp)
                    p = ep.tile([P, L], f16, tag="ep", name="p_t")
                    nc.vector.tensor_tensor(
                        out=p[:, :], in0=e[:, :],
                        in1=t_expBT[:, kc * L:(kc + 1) * L], op=Alu.mult)
                    for qc in range(NQ):
                        ab, j = divmod(qc, 4)
                        nc.tensor.matmul(
                            pso[ab][:, j * W65:(j + 1) * W65],
                            lhsT=p[:, qc * P:(qc + 1) * P],
                            rhs=t_vaug[kc][:, h * W65:(h + 1) * W65],
                            start=(kc == 0 and j == 0),
                            stop=(kc == NK - 1 and j == 3),
                            skip_group_check=(
                                j != 0 and not (kc == NK - 1 and j == 3)))
                for ab in range(2):
                    po = pso[ab]
                    r4 = small.tile([P, 4], f32, tag="r", name="r_t")
                    nc.vector.reciprocal(
                        out=r4[:, :],
                        in_=po[:, :].rearrange("p (j c) -> p j c", c=W65)
                        [:, :, DK:DK + 1].rearrange("p j c -> p (j c)"))
                    for j in range(4):
                        qc = ab * 4 + j
                        nc.vector.tensor_scalar_mul(
                            t_ctx[qc][:, h * DK:(h + 1) * DK],
                            po[:, j * W65:j * W65 + DK], r4[:, j:j + 1])
                if h % 2 == 1:
                    pending_ct.append(ht)

            for pt in pending_ct:
                for qc in range(NQ):
                    pct = ps_att.tile([P, P], f16, tag="ps_att", name="pct")
                    nc.tensor.transpose(
                        out=pct[:, :],
                        in_=t_ctx[qc][:, pt * P:(pt + 1) * P],
                        identity=t_identh[:, :])
                    nc.vector.tensor_copy(
                        t_cT[qc][:, pt * P:(pt + 1) * P], pct[:, :])

            # ---- output projection ----
            for qc in range(NQ):
                psf = ps_mm.tile([P, D], f32, tag="ps_mm", name="ps_f")
                for dc in range(KC):
                    nc.tensor.matmul(
                        psf[:, :],
                        lhsT=t_cT[qc][:, dc * P:(dc + 1) * P],
                        rhs=t_woT[dc][:, :],
                        start=(dc == 0), stop=(dc == KC - 1))
                ot = outp.tile([P, D], f32, tag="ot", name="ot")
                if zb:
                    nc.vector.tensor_copy(ot[:, :], psf[:, :])
                else:
                    nc.vector.tensor_tensor(
                        out=ot[:, :], in0=psf[:, :], in1=t_bob[:, :],
                        op=Alu.add)
                nc.sync.dma_start(out=out[qc * P:(qc + 1) * P, :], in_=ot[:, :])

    nc.compile()
    return nc


def _get_nc(zb):
    key = f"nc_{zb}"
    if key not in _CACHE:
        _CACHE[key] = _build(zb)
    return _CACHE[key]


def _in_maps(inputs):
    q = np.asarray(inputs["query"], np.float32)
    k = np.asarray(inputs["key"], np.float32)
    v = np.asarray(inputs["value"], np.float32)
    mask = np.asarray(inputs["mask"], np.int32).reshape(B, L, L)
    tree = np.asarray(inputs["tree_score"], np.float32)
    leaf = np.asarray(inputs["leaf_score"], np.float32)
    shared = {
        "wqT": np.ascontiguousarray(np.asarray(inputs["Wq"], np.float32).T.astype(np.float16)),
        "wkT": np.ascontiguousarray(np.asarray(inputs["Wk"], np.float32).T.astype(np.float16)),
        "wvT": np.ascontiguousarray(np.asarray(inputs["Wv"], np.float32).T.astype(np.float16)),
        "woT": np.ascontiguousarray(np.asarray(inputs["Wo"], np.float32).T.astype(np.float16)),
        "bqc": np.asarray(inputs["bq"], np.float32).reshape(D, 1),
        "bkc": np.asarray(inputs["bk"], np.float32).reshape(D, 1),
        "bv": np.asarray(inputs["bv"], np.float32).reshape(1, D).astype(np.float16),
        "bo": np.asarray(inputs["bo"], np.float32).reshape(1, D),
        "identh": np.eye(P, dtype=np.float16),
        "ones": np.ones((1, D), np.float16),
    }
    maps = []
    for b in range(B):
        m = dict(shared)
        m["xqT"] = np.ascontiguousarray(q[b].T.astype(np.float16))
        m["xkT"] = np.ascontiguousarray(k[b].T.astype(np.float16))
        m["xvT"] = np.ascontiguousarray(v[b].T.astype(np.float16))
        m["mask"] = mask[b].astype(np.int8)
        m["tree"] = tree[b].astype(np.float16)
        m["leaf"] = leaf[b].astype(np.float16)
        maps.append(m)
    return maps


def _run(inputs, **kw):
    from concourse.bass_utils import run_bass_kernel_spmd
    zb = not (np.any(np.asarray(inputs["bq"])) or np.any(np.asarray(inputs["bk"]))
              or np.any(np.asarray(inputs["bv"])) or np.any(np.asarray(inputs["bo"])))
    nc = _get_nc(zb)
    res = run_bass_kernel_spmd(nc, _in_maps(inputs), core_ids=list(range(B)), **kw)
    out = np.stack([res.results[b]["out"] for b in range(B)])
    return out, res


def kernel(**inputs):
    out, _ = _run(inputs)
    return out


# revision 32
# speedup vs baseline: 1.0094x; 1.0094x over previous
# Multi-headed attention with additive tree/leaf score biases, on 8 TRN2
# NeuronCores. Data-parallel over batch: core b computes batch element b.
#
# Per-core dataflow (L=1024, D=512, H=8, dk=64), fp16 matmul operands with
# fp32 PSUM accumulation:
#   QTz_h[d,l]: zero-padded per-head Q^T (head rows live, other half 0) so
#       the QK matmul runs K=128 (fast-weight-load path) with the packed
#       two-head K^T tile as stationary operand.
#   ST[kpos,q] = KT_pair^T @ QTz_h    (scores transposed, pre-scaled 1/8)
#   e = exp(ST)  (ACT, psum->sbuf, fp16)
#   p = e * expBT,  expBT = exp(tree+leaf+maskbias - C)^T  (fp16, C=offset
#       keeping p in fp16 range; cancels in softmax normalization)
#   U[q, 0:64|64] = p^T @ [V_h | 1]   (col 64 = softmax denominator s)
#   ctx[q, 64h:64h+64] = U[:, 0:64] * (1/s)   (per-partition scalar)
#   out = ctx @ Wo^T (+bo)  via ctx^T on-device transpose
import numpy as np

B, L, D, H = 8, 1024, 512, 8
DK = D // H          # 64
P = 128
NQ = L // P          # 8 q-chunks
NK = L // P          # 8 kpos-chunks
KC = D // P          # 4 contraction chunks
MASK_C = 60.0        # masked-score offset (expB underflows to 0 in fp16)
EXPB_C = 6.931472    # ~10*ln(2): headroom so p = e*expB stays in fp16 range

_CACHE = {}


def _build(zb):
    import concourse.mybir as mybir
    import concourse.tile as tile
    from concourse import bacc

    f32 = mybir.dt.float32
    f16 = mybir.dt.float16
    i8 = mybir.dt.int8
    Exp = mybir.ActivationFunctionType.Exp
    Copy = mybir.ActivationFunctionType.Copy
    Ident = mybir.ActivationFunctionType.Identity
    Alu = mybir.AluOpType

    nc = bacc.Bacc("TRN2", target_bir_lowering=False, debug=False)

    xqT = nc.declare_dram_parameter("xqT", [D, L], f16, isOutput=False)
    xkT = nc.declare_dram_parameter("xkT", [D, L], f16, isOutput=False)
    xvT = nc.declare_dram_parameter("xvT", [D, L], f16, isOutput=False)
    wqT = nc.declare_dram_parameter("wqT", [D, D], f16, isOutput=False)
    wkT = nc.declare_dram_parameter("wkT", [D, D], f16, isOutput=False)
    wvT = nc.declare_dram_parameter("wvT", [D, D], f16, isOutput=False)
    woT = nc.declare_dram_parameter("woT", [D, D], f16, isOutput=False)
    bqc = nc.declare_dram_parameter("bqc", [D, 1], f32, isOutput=False)
    bkc = nc.declare_dram_parameter("bkc", [D, 1], f32, isOutput=False)
    bvd = nc.declare_dram_parameter("bv", [1, D], f16, isOutput=False)
    bodf = nc.declare_dram_parameter("bo", [1, D], f32, isOutput=False)
    mask = nc.declare_dram_parameter("mask", [L, L], i8, isOutput=False)
    tree = nc.declare_dram_parameter("tree", [L, L], f16, isOutput=False)
    leaf = nc.declare_dram_parameter("leaf", [L, L], f16, isOutput=False)
    identh = nc.declare_dram_parameter("identh", [P, P], f16, isOutput=False)
    ones = nc.declare_dram_parameter("ones", [1, D], f16, isOutput=False)
    out = nc.declare_dram_parameter("out", [L, D], f32, isOutput=True)

    with tile.TileContext(nc) as tc:
        import contextlib
        ctx = contextlib.ExitStack()
        with ctx:
            consts = ctx.enter_context(tc.tile_pool(name="consts", bufs=1))
            persist = ctx.enter_context(tc.tile_pool(name="persist", bufs=1))
            wpool = ctx.enter_context(tc.tile_pool(name="wpool", bufs=12))
            xpool = ctx.enter_context(tc.tile_pool(name="xpool", bufs=12))
            ep = ctx.enter_context(tc.tile_pool(name="ep", bufs=17))
            big4 = ctx.enter_context(tc.tile_pool(name="big4", bufs=4))
            bnat = ctx.enter_context(tc.tile_pool(name="bnat", bufs=2))
            small = ctx.enter_context(tc.tile_pool(name="small", bufs=4))
            outp = ctx.enter_context(tc.tile_pool(name="outp", bufs=3))
            ps_mm = ctx.enter_context(
                tc.tile_pool(name="ps_mm", bufs=3, space="PSUM"))
            ps_att = ctx.enter_context(
                tc.tile_pool(name="ps_att", bufs=2, space="PSUM"))

            # ---- constants ----
            t_identh = consts.tile([P, P], f16)
            nc.sync.dma_start(out=t_identh[:, :], in_=identh[:, :])
            t_ones = consts.tile([1, D], f16)
            nc.sync.dma_start(out=t_ones[:, :], in_=ones[:, :])
            if not zb:
                t_bvb = consts.tile([P, D], f16)
                nc.sync.dma_start(out=t_bvb[:, :],
                                  in_=bvd[:, :].broadcast_to([P, D]))
                t_bob = consts.tile([P, D], f32)
                nc.sync.dma_start(out=t_bob[:, :],
                                  in_=bodf[:, :].broadcast_to([P, D]))
                # per-partition bias columns for the Q/K projection evacs;
                # bq pre-divided by 8 (the evac applies scale=1/8 to psum)
                t_bqc, t_bkc = [], []
                for dc in range(KC):
                    tq = consts.tile([P, 1], f32, name=f"bqc_{dc}")
                    nc.sync.dma_start(out=tq[:, :],
                                      in_=bqc[dc * P:(dc + 1) * P, :])
                    tq8 = consts.tile([P, 1], f32, name=f"bqc8_{dc}")
                    nc.vector.tensor_scalar_mul(tq8[:, :], tq[:, :], 1.0 / 8.0)
                    t_bqc.append(tq8)
                    tk = consts.tile([P, 1], f32, name=f"bkc_{dc}")
                    nc.sync.dma_start(out=tk[:, :],
                                      in_=bkc[dc * P:(dc + 1) * P, :])
                    t_bkc.append(tk)

            # ---- weights/activations arrive fp16 from host ----
            def load_w_persist(dram):
                ts = []
                for kc in range(KC):
                    th = persist.tile([P, D], f16, tag=f"woT_{kc}",
                                      name=f"wh_{dram.name}_{kc}")
                    nc.sync.dma_start(
                        out=th[:, :], in_=dram[kc * P:(kc + 1) * P, :])
                    ts.append(th)
                return ts

            def load_w(dram, pool, tag):
                ts = []
                for kc in range(KC):
                    th = pool.tile([P, D], f16, tag=tag,
                                   name=f"wh_{dram.name}_{kc}")
                    nc.sync.dma_start(
                        out=th[:, :], in_=dram[kc * P:(kc + 1) * P, :])
                    ts.append(th)
                return ts

            def load_xT(dram):
                ts = []
                for kc in range(KC):
                    th = xpool.tile([P, L], f16, tag="xp",
                                    name=f"xh_{dram.name}_{kc}")
                    nc.sync.dma_start(
                        out=th[:, :], in_=dram[kc * P:(kc + 1) * P, :])
                    ts.append(th)
                return ts

            # ---- bias matrix pipeline (after QK: attention needs it a bit later):
            #      B = tree + leaf + (mask==0 ? -MASK_C : 0) - EXPB_C
            #      expBT_all[:, kc*L + q] = exp(B)[q, kc*128 + p]  (fp16) ----
            t_expBT = persist.tile([P, NK * L], f16, tag="expBT", name="expBT")
            for qc in range(NQ):
                bn = bnat.tile([P, L], f16, tag="bn", name="bn")
                nc.sync.dma_start(out=bn[:, :], in_=tree[qc * P:(qc + 1) * P, :])
                lf = bnat.tile([P, L], f16, tag="lf", name="lf")
                nc.sync.dma_start(out=lf[:, :], in_=leaf[qc * P:(qc + 1) * P, :])
                mk = bnat.tile([P, L], i8, tag="mk", name="mk")
                nc.sync.dma_start(out=mk[:, :], in_=mask[qc * P:(qc + 1) * P, :])
                mb = bnat.tile([P, L], f16, tag="mb", name="mb")
                nc.gpsimd.tensor_scalar(
                    out=mb[:, :], in0=mk[:, :], scalar1=MASK_C,
                    scalar2=-(MASK_C + EXPB_C), op0=Alu.mult, op1=Alu.add)
                bs = bnat.tile([P, L], f16, tag="bs", name="bs")
                nc.vector.tensor_tensor(
                    out=bs[:, :], in0=bn[:, :], in1=lf[:, :], op=Alu.add)
                nc.vector.tensor_tensor(
                    out=bs[:, :], in0=bs[:, :], in1=mb[:, :], op=Alu.add)
                eb = bnat.tile([P, L], f16, tag="eb", name="eb")
                nc.scalar.activation(out=eb[:, :], in_=bs[:, :], func=Exp)
                # transpose eb into column-slab qc of expBT (per kc chunk)
                pst = ps_mm.tile([P, L], f16, tag="ps_mm", name="ps_bt")
                for kc in range(NK):
                    nc.tensor.transpose(
                        out=pst[:, kc * P:(kc + 1) * P],
                        in_=eb[:, kc * P:(kc + 1) * P],
                        identity=t_identh[:, :])
                nc.vector.tensor_copy(
                    t_expBT[:, :].rearrange("p (kc q) -> p kc q", q=L)
                    [:, :, qc * P:(qc + 1) * P],
                    pst[:, :].rearrange("p (kc c) -> p kc c", c=P))


            t_wqT = load_w(wqT, wpool, "wp")
            t_wkT = load_w(wkT, wpool, "wp")
            t_xqT = load_xT(xqT)
            t_xkT = load_xT(xkT)

            # ---- Q/K projections.  K^T packed two heads per tile;
            #      Q^T zero-padded per head (K=128 QK matmuls -> FWL). ----
            t_QTz = []
            for h in range(H):
                qz = persist.tile([P, L], f16, tag=f"qTz_{h}", name=f"qTz_{h}")
                t_QTz.append(qz)
                if h % 2 == 0:
                    nc.gpsimd.memset(qz[DK:P, :], 0.0)
                else:
                    nc.gpsimd.memset(qz[0:DK, :], 0.0)
            t_KT = []
            for dc in range(KC):
                kt = persist.tile([P, L], f16, tag=f"kT_{dc}", name=f"kT_{dc}")
                t_KT.append(kt)
                for lh in range(2):
                    ps = ps_mm.tile([P, D], f32, tag="ps_mm", name="ps_k")
                    for kc in range(KC):
                        nc.tensor.matmul(
                            ps[:, :],
                            lhsT=t_wkT[kc][:, dc * P:(dc + 1) * P],
                            rhs=t_xkT[kc][:, lh * D:(lh + 1) * D],
                            start=(kc == 0), stop=(kc == KC - 1))
                    if zb:
                        nc.vector.tensor_copy(
                            kt[:, lh * D:(lh + 1) * D], ps[:, :])
                    else:
                        nc.scalar.activation(
                            out=kt[:, lh * D:(lh + 1) * D], in_=ps[:, :],
                            func=Ident, bias=t_bkc[dc][:, :])
            for dc in range(KC):
                for lh in range(2):
                    ps = ps_mm.tile([P, D], f32, tag="ps_mm", name="ps_q")
                    for kc in range(KC):
                        nc.tensor.matmul(
                            ps[:, :],
                            lhsT=t_wqT[kc][:, dc * P:(dc + 1) * P],
                            rhs=t_xqT[kc][:, lh * D:(lh + 1) * D],
                            start=(kc == 0), stop=(kc == KC - 1))
                    # psum rows 0:64 -> head 2dc, rows 64:128 -> head 2dc+1
                    if zb:
                        nc.scalar.activation(
                            out=t_QTz[2 * dc][0:DK, lh * D:(lh + 1) * D],
                            in_=ps[0:DK, :], func=Copy, scale=1.0 / 8.0)
                        nc.scalar.activation(
                            out=t_QTz[2 * dc + 1][DK:P, lh * D:(lh + 1) * D],
                            in_=ps[DK:P, :], func=Copy, scale=1.0 / 8.0)
                    else:
                        nc.scalar.activation(
                            out=t_QTz[2 * dc][0:DK, lh * D:(lh + 1) * D],
                            in_=ps[0:DK, :], func=Ident, scale=1.0 / 8.0,
                            bias=t_bqc[dc][0:DK, :])
                        nc.scalar.activation(
                            out=t_QTz[2 * dc + 1][DK:P, lh * D:(lh + 1) * D],
                            in_=ps[DK:P, :], func=Ident, scale=1.0 / 8.0,
                            bias=t_bqc[dc][DK:P, :])

            # ---- Xv^T + V projection -> V_aug [128, 520] fp16 x 8 ----
            t_wvT = load_w(wvT, wpool, "wp")
            t_xvT = load_xT(xvT)
            t_woT = load_w_persist(woT)
            t_vaug = []
            for lc in range(NK):
                va = persist.tile([P, H * (DK + 1)], f16, tag=f"vaug_{lc}",
                                  name=f"vaug_{lc}")
                t_vaug.append(va)
                nc.gpsimd.memset(va[:, :], 1.0)
                ps = ps_mm.tile([P, D], f32, tag="ps_mm", name="ps_v")
                for kc in range(KC):
                    nc.tensor.matmul(
                        ps[:, :],
                        lhsT=t_xvT[kc][:, lc * P:(lc + 1) * P],
                        rhs=t_wvT[kc][:, :],
                        start=(kc == 0), stop=(kc == KC - 1))
                if zb:
                    nc.vector.tensor_copy(
                        va[:, :].rearrange("p (h c) -> p h c", c=DK + 1)[:, :, 0:DK],
                        ps[:, :].rearrange("p (h c) -> p h c", c=DK))
                else:
                    nc.scalar.activation(
                        out=va[:, :].rearrange("p (h c) -> p h c", c=DK + 1)[:, :, 0:DK],
                        in_=ps[:, :].rearrange("p (h c) -> p h c", c=DK),
                        func=Copy)
                    nc.vector.tensor_tensor(
                        out=va[:, :].rearrange("p (h c) -> p h c", c=DK + 1)[:, :, 0:DK],
                        in0=va[:, :].rearrange("p (h c) -> p h c", c=DK + 1)[:, :, 0:DK],
                        in1=t_bvb[:, :].rearrange("p (h c) -> p h c", c=DK),
                        op=Alu.add)

            # ---- attention: per head, kc-incremental attnV ----
            t_ctx = [persist.tile([P, D], f16, tag=f"ctx_{qc}",
                                  name=f"ctx_{qc}") for qc in range(NQ)]
            t_cT = [persist.tile([P, D], f16, tag=f"cT_{qc}",
                                 name=f"cT_{qc}") for qc in range(NQ)]
            t_acc = []
            pending_ct = []
            W65 = DK + 1
            for h in range(H):
                ht = h // 2
                pso = [ps_att.tile([P, 4 * W65], f32, tag="ps_att", name="pso")
                       for _ in range(2)]
                for kc in range(NK):
                    ps = ps_mm.tile([P, L], f32, tag="ps_mm", name="ps_sc")
                    for qh in range(2):
                        nc.tensor.matmul(
                            ps[:, qh * D:(qh + 1) * D],
                            lhsT=t_KT[ht][:, kc * P:(kc + 1) * P],
                            rhs=t_QTz[h][:, qh * D:(qh + 1) * D],
                            start=True, stop=True)
                    e = ep.tile([P, L], f16, tag="ep", name="e_t")
                    nc.scalar.activation(out=e[:, :], in_=ps[:, :], func=Exp)
                    p = ep.tile([P, L], f16, tag="ep", name="p_t")
                    nc.vector.tensor_tensor(
                        out=p[:, :], in0=e[:, :],
                        in1=t_expBT[:, kc * L:(kc + 1) * L], op=Alu.mult)
                    for qc in range(NQ):
                        ab, j = divmod(qc, 4)
                        nc.tensor.matmul(
                            pso[ab][:, j * W65:(j + 1) * W65],
                            lhsT=p[:, qc * P:(qc + 1) * P],
                            rhs=t_vaug[kc][:, h * W65:(h + 1) * W65],
                            start=(kc == 0 and j == 0),
                            stop=(kc == NK - 1 and j == 3),
                            skip_group_check=(
                                j != 0 and not (kc == NK - 1 and j == 3)))
                for ab in range(2):
                    po = pso[ab]
                    r4 = small.tile([P, 4], f32, tag="r", name="r_t")
                    nc.vector.reciprocal(
                        out=r4[:, :],
                        in_=po[:, :].rearrange("p (j c) -> p j c", c=W65)
                        [:, :, DK:DK + 1].rearrange("p j c -> p (j c)"))
                    for j in range(4):
                        qc = ab * 4 + j
                        nc.vector.tensor_scalar_mul(
                            t_ctx[qc][:, h * DK:(h + 1) * DK],
                            po[:, j * W65:j * W65 + DK], r4[:, j:j + 1])
                if h % 2 == 1:
                    for qc in range(NQ):
                        pct = ps_att.tile([P, P], f16, tag="ps_att",
                                          name="pct")
                        nc.tensor.transpose(
                            out=pct[:, :],
                            in_=t_ctx[qc][:, ht * P:(ht + 1) * P],
                            identity=t_identh[:, :])
                        nc.vector.tensor_copy(
                            t_cT[qc][:, ht * P:(ht + 1) * P], pct[:, :])

            # ---- output projection ----
            for qc in range(NQ):
                psf = ps_mm.tile([P, D], f32, tag="ps_mm", name="ps_f")
                for dc in range(KC):
                    nc.tensor.matmul(
                        psf[:, :],
                        lhsT=t_cT[qc][:, dc * P:(dc + 1) * P],
                        rhs=t_woT[dc][:, :],
                        start=(dc == 0), stop=(dc == KC - 1))
                ot = outp.tile([P, D], f32, tag="ot", name="ot")
                if zb:
                    nc.vector.tensor_copy(ot[:, :], psf[:, :])
                else:
                    nc.vector.tensor_tensor(
                        out=ot[:, :], in0=psf[:, :], in1=t_bob[:, :],
                        op=Alu.add)
                nc.sync.dma_start(out=out[qc * P:(qc + 1) * P, :], in_=ot[:, :])

    nc.compile()
    return nc


def _get_nc(zb):
    key = f"nc_{zb}"
    if key not in _CACHE:
        _CACHE[key] = _build(zb)
    return _CACHE[key]


def _in_maps(inputs):
    q = np.asarray(inputs["query"], np.float32)
    k = np.asarray(inputs["key"], np.float32)
    v = np.asarray(inputs["value"], np.float32)
    mask = np.asarray(inputs["mask"], np.int32).reshape(B, L, L)
    tree = np.asarray(inputs["tree_score"], np.float32)
    leaf = np.asarray(inputs["leaf_score"], np.float32)
    shared = {
        "wqT": np.ascontiguousarray(np.asarray(inputs["Wq"], np.float32).T.astype(np.float16)),
        "wkT": np.ascontiguousarray(np.asarray(inputs["Wk"], np.float32).T.astype(np.float16)),
        "wvT": np.ascontiguousarray(np.asarray(inputs["Wv"], np.float32).T.astype(np.float16)),
        "woT": np.ascontiguousarray(np.asarray(inputs["Wo"], np.float32).T.astype(np.float16)),
        "bqc": np.asarray(inputs["bq"], np.float32).reshape(D, 1),
        "bkc": np.asarray(inputs["bk"], np.float32).reshape(D, 1),
        "bv": np.asarray(inputs["bv"], np.float32).reshape(1, D).astype(np.float16),
        "bo": np.asarray(inputs["bo"], np.float32).reshape(1, D),
        "identh": np.eye(P, dtype=np.float16),
        "ones": np.ones((1, D), np.float16),
    }
    maps = []
    for b in range(B):
        m = dict(shared)
        m["xqT"] = np.ascontiguousarray(q[b].T.astype(np.float16))
        m["xkT"] = np.ascontiguousarray(k[b].T.astype(np.float16))
        m["xvT"] = np.ascontiguousarray(v[b].T.astype(np.float16))
        m["mask"] = mask[b].astype(np.int8)
        m["tree"] = tree[b].astype(np.float16)
        m["leaf"] = leaf[b].astype(np.float16)
        maps.append(m)
    return maps


def _run(inputs, **kw):
    from concourse.bass_utils import run_bass_kernel_spmd
    zb = not (np.any(np.asarray(inputs["bq"])) or np.any(np.asarray(inputs["bk"]))
              or np.any(np.asarray(inputs["bv"])) or np.any(np.asarray(inputs["bo"])))
    nc = _get_nc(zb)
    res = run_bass_kernel_spmd(nc, _in_maps(inputs), core_ids=list(range(B)), **kw)
    out = np.stack([res.results[b]["out"] for b in range(B)])
    return out, res


def kernel(**inputs):
    out, _ = _run(inputs)
    return out
